# revision 1
# baseline (speedup 1.0000x reference)
"""Trainium2 Bass kernel for nn_MemoryAttention (causal single-head attention
with SiLU-gated output projection), sequence-parallel across 8 NeuronCores.

Strategy (per core c):
  - q rows owned: 4 slots of 256 rows: tile t = c + 8*s (strided assignment
    balances causal work; every core runs an identical instruction stream).
  - QT/KT computed in [d, s] layout (contraction dim on partitions), V in
    natural [s, d]. Each core projects KT/V for its own rows, AllGathers
    them in bf16, and locally duplicates the first B_DUP kv blocks to hide
    the collective's latency.
  - Per (slot, kv-block) visit: LT[kv, q] = K @ QT accumulated in PSUM
    (lhsT = KT subtiles), PT = exp(LT/32) (* mask for the last 16 visits of
    each slot; mask tensors streamed per-core keep the instruction stream
    uniform), then PT q-chunks become the stationary operand for both
    H[q, d] += P @ V (N=512) and rowsums += P @ 1 (N=1, shares the LDW).
  - Slot epilogue: H / sums (per-partition scalar), SiLU, PE-transpose of G,
    output projection with G^T chunks stationary -> O[q, d] written directly.
"""

import numpy as np
import ml_dtypes

import concourse.bass as bass
import concourse.tile as tile
from concourse import bacc, mybir
from concourse.bass_utils import run_bass_kernel_spmd
from concourse.masks import make_identity

P = 128
D = 1024
SEQ = 8192
NCORES = 8
NSLOTS = 4
QT_COLS = NSLOTS * 256
B_DUP = 8
N_MASKED = NSLOTS * 16  # visits with j >= 16*s need a mask on some core

F32 = mybir.dt.float32
BF16 = mybir.dt.bfloat16
AF = mybir.ActivationFunctionType


def build_kernel(b_dup=B_DUP):
    assert b_dup % 4 == 0, "kt_proj writes kv_dup in 512-column chunks"
    nc = bacc.Bacc(None, target_bir_lowering=False, num_devices=NCORES)

    xq_ext = nc.declare_dram_parameter("xq", [D, QT_COLS], BF16, isOutput=False)
    xd_ext = nc.declare_dram_parameter("xd", [D, b_dup * P], BF16, isOutput=False)
    wq_ext = nc.declare_dram_parameter("wq", [D, D], BF16, isOutput=False)
    wk_ext = nc.declare_dram_parameter("wk", [D, D], BF16, isOutput=False)
    wv1_ext = nc.declare_dram_parameter("wv1", [D, D], BF16, isOutput=False)
    wv2_ext = nc.declare_dram_parameter("wv2", [D, D], BF16, isOutput=False)
    mask_ext = nc.declare_dram_parameter("masks", [N_MASKED, P, 256], BF16, isOutput=False)
    o_ext = nc.declare_dram_parameter("o", [NSLOTS, 2, P, D], F32, isOutput=True)

    # blocked kv payloads: [grp][kind(kt=0,v=1)][slot-in-grp][half][128][8][128]
    # split into two slot-group collectives so blocks 0-31 arrive early
    kv_local = nc.dram_tensor("kv_local", [2, 2, 2, 2, P, 8, P], BF16)
    kv_gathA = nc.dram_tensor(
        "kv_gathA", [NCORES, 2, 2, 2, P, 8, P], BF16, addr_space="Shared"
    )
    kv_gathB = nc.dram_tensor(
        "kv_gathB", [NCORES, 2, 2, 2, P, 8, P], BF16, addr_space="Shared"
    )
    kv_dup = nc.dram_tensor("kv_dup", [b_dup, 2, P, 8, P], BF16)

    def wload(nc, pool, ext, tag):
        t = pool.tile([P, 8, D], BF16, tag=tag, name=tag)
        nc.sync.dma_start(out=t, in_=ext[:].rearrange("(sub p) s -> p sub s", p=P))
        return t

    with tile.TileContext(nc) as tc:
        singles_ctx = tc.tile_pool(name="singles", bufs=1)
        singles = singles_ctx.__enter__()

        with (
            tc.tile_pool(name="projw", bufs=1) as projw,
            tc.tile_pool(name="xstream", bufs=2) as xstream,
            tc.tile_pool(name="projout", bufs=4) as projout,
            tc.tile_pool(name="ppsum", bufs=4, space="PSUM") as ppsum,
        ):
            # chunked loads so the first projection matmuls start early
            wk_bf = projw.tile([P, 8, D], BF16, tag="wk", name="wk")
            wk_v = wk_ext[:].rearrange("(sub p) s -> p sub s", p=P)
            xq_bf = singles.tile([P, 8, QT_COLS], BF16)
            xq_v = xq_ext[:].rearrange("(sub p) s -> p sub s", p=P)
            nc.sync.dma_start(out=xq_bf[:, :, :512], in_=xq_v[:, :, :512])
            for m in range(8):
                nc.sync.dma_start(
                    out=wk_bf[:, :, m * P : (m + 1) * P],
                    in_=wk_v[:, :, m * P : (m + 1) * P],
                )
            nc.sync.dma_start(out=xq_bf[:, :, 512:], in_=xq_v[:, :, 512:])
            wv1_bf = wload(nc, projw, wv1_ext, "wv1")

            ones_sb = singles.tile([P, 1], BF16)
            nc.vector.memset(ones_sb, 1.0)
            zcol_sb = singles.tile([1, P], BF16)
            nc.vector.memset(zcol_sb, 0.0)
            zrow_sb = singles.tile([1, 512], BF16)
            nc.vector.memset(zrow_sb, 0.0)
            ident_sb = singles.tile([P, P], BF16)
            make_identity(nc, ident_sb)

            def kt_proj(dst, w_bf, src_bf, col0, col1):
                # dst(blk)[0] <- KT payload: [p(dout), m, c]
                # sub outer / n inner so consecutive matmuls share lhsT
                chunks = list(range(col0 // 512, col1 // 512))
                for m in range(8):
                    accs = [
                        ppsum.tile([P, 512], F32, tag="proj", name=f"ktp{i}")
                        for i in range(len(chunks))
                    ]
                    for sub in range(8):
                        for i, n in enumerate(chunks):
                            nc.tensor.matmul(
                                accs[i],
                                lhsT=w_bf[:, sub, m * P : (m + 1) * P],
                                rhs=src_bf[:, sub, n * 512 : (n + 1) * 512],
                                start=(sub == 0),
                                stop=(sub == 7),
                            )
                    for i, n in enumerate(chunks):
                        kt_out = projout.tile([P, 512], BF16, tag="kt_out", name="kto")
                        nc.vector.tensor_copy(out=kt_out, in_=accs[i])
                        for b in range(4):
                            dst_ap = dst(n * 4 + b)
                            nc.sync.dma_start(
                                out=dst_ap[0, :, m, :],
                                in_=kt_out[:, b * P : (b + 1) * P],
                            )

            def v_proj(dst, wv_bf, src_bf, col0, col1):
                for blk in range(col0 // P, col1 // P):
                    v_out = projout.tile([P, 1024], BF16, tag="v_out", name="vo")
                    accs = [
                        ppsum.tile([P, 512], F32, tag="proj", name=f"vp{h2}")
                        for h2 in range(2)
                    ]
                    for sub in range(8):
                        for h2 in range(2):
                            nc.tensor.matmul(
                                accs[h2],
                                lhsT=src_bf[:, sub, blk * P : (blk + 1) * P],
                                rhs=wv_bf[:, sub, h2 * 512 : (h2 + 1) * 512],
                                start=(sub == 0),
                                stop=(sub == 7),
                            )
                    for h2 in range(2):
                        nc.vector.tensor_copy(
                            out=v_out[:, h2 * 512 : (h2 + 1) * 512], in_=accs[h2]
                        )
                    nc.sync.dma_start(
                        out=dst(blk)[1].rearrange("p m c -> p (m c)"), in_=v_out
                    )

            # ---- own KT/V -> kv_local; gather each slot-group asap -------
            own_dst = lambda blk: kv_local[blk // 4, :, (blk // 2) % 2, blk % 2]
            for grp, gath in ((0, kv_gathA), (1, kv_gathB)):
                kt_proj(own_dst, wk_bf, xq_bf, grp * 512, (grp + 1) * 512)
                v_proj(own_dst, wv1_bf, xq_bf, grp * 512, (grp + 1) * 512)
                nc.gpsimd.collective_compute(
                    "AllGather",
                    mybir.AluOpType.bypass,
                    replica_groups=[list(range(NCORES))],
                    ins=[kv_local[grp]],
                    outs=[gath[:]],
                )

            # ---- QT -------------------------------------------------------
            wq_bf = wload(nc, projw, wq_ext, "wq")
            qt_sb = singles.tile([P, 8, QT_COLS], BF16)
            for m in range(8):
                accs = [
                    ppsum.tile([P, 512], F32, tag="proj", name=f"qp{n}")
                    for n in range(2)
                ]
                for sub in range(8):
                    for n in range(2):
                        nc.tensor.matmul(
                            accs[n],
                            lhsT=wq_bf[:, sub, m * P : (m + 1) * P],
                            rhs=xq_bf[:, sub, n * 512 : (n + 1) * 512],
                            start=(sub == 0),
                            stop=(sub == 7),
                        )
                for n in range(2):
                    nc.vector.tensor_copy(
                        out=qt_sb[:, m, n * 512 : (n + 1) * 512], in_=accs[n]
                    )

            # ---- duplicated kv prefix ------------------------------------
            if b_dup:
                xd_bf = xstream.tile([P, 8, b_dup * P], BF16, tag="xd", name="xd")
                nc.sync.dma_start(
                    out=xd_bf, in_=xd_ext[:].rearrange("(sub p) s -> p sub s", p=P)
                )
                dup_dst = lambda blk: kv_dup[blk]
                kt_proj(dup_dst, wk_bf, xd_bf, 0, b_dup * P)
                v_proj(dup_dst, wv1_bf, xd_bf, 0, b_dup * P)

        # ---- attention ----------------------------------------------------
        with (
            tc.tile_pool(name="asingles", bufs=1) as asingles,
            tc.tile_pool(name="vpool", bufs=10) as vpool,
            tc.tile_pool(name="mpool", bufs=3) as mpool,
            tc.tile_pool(name="epool", bufs=2) as epool,
            tc.tile_pool(name="gpool", bufs=2) as gpool,
            tc.tile_pool(name="ltpsum", bufs=2, space="PSUM") as ltpsum,
            tc.tile_pool(name="hpsum", bufs=1, space="PSUM") as hpsum,
            tc.tile_pool(name="spsum", bufs=1, space="PSUM") as spsum,
            tc.tile_pool(name="tppsum", bufs=1, space="PSUM") as tppsum,
        ):
            wv2_bf = wload(nc, asingles, wv2_ext, "wv2")

            def visit_srcs(s, j):
                if j < b_dup:
                    base = kv_dup[j]
                else:
                    t = j // 2
                    s_own = t // 8
                    gath = kv_gathA if s_own < 2 else kv_gathB
                    base = gath[t % 8, :, s_own % 2, j % 2]
                return base[0], base[1].rearrange("p m c -> p (m c)")

            def load_visit(s, j):
                # gpsimd queue: keeps visit streams off the sync queue that
                # carries kv_local writes + mask loads (head-of-line blocking)
                kt_src, v_src = visit_srcs(s, j)
                kt_t = vpool.tile([P, 8, P], BF16, tag="kt", name="kt_t")
                nc.sync.dma_start(out=kt_t, in_=kt_src)
                v_t = vpool.tile([P, 1024], BF16, tag="v", name="v_t")
                nc.sync.dma_start(out=v_t, in_=v_src)
                return kt_t, v_t

            def logits(s, j, kt_t):
                lt = ltpsum.tile([P, 256], F32, tag="lt", name="lt")
                for sub in range(8):
                    nc.tensor.matmul(
                        lt,
                        lhsT=kt_t[:, sub, :],
                        rhs=qt_sb[:, sub, s * 256 : (s + 1) * 256],
                        start=(sub == 0),
                        stop=(sub == 7),
                    )
                return lt

            def pv(s, j, lt, v_t, h, sums, jmax):
                pt = vpool.tile([P, 256], BF16, tag="pt", name="pt")
                nc.scalar.activation(out=pt, in_=lt, func=AF.Exp, scale=0.03125)
                if j >= 16 * s:
                    m_t = mpool.tile([P, 256], BF16, tag="m", name="m_t")
                    nc.sync.dma_start(out=m_t, in_=mask_ext[16 * s + (j - 16 * s)])
                    nc.vector.tensor_mul(out=pt, in0=pt, in1=m_t)
                for qc in range(2):
                    lhsT = pt[:, qc * P : (qc + 1) * P]
                    for dh in range(2):
                        nc.tensor.matmul(
                            h[qc][:, dh, :],
                            lhsT=lhsT,
                            rhs=v_t[:, dh * 512 : (dh + 1) * 512],
                            start=(j == 0),
                            stop=(j == jmax),
                        )
                    nc.tensor.matmul(
                        sums[:, qc : qc + 1],
                        lhsT=lhsT,
                        rhs=ones_sb,
                        start=False,
                        stop=(j == jmax),
                        skip_group_check=True,
                    )

            for s in range(NSLOTS):
                nv = 16 * (s + 1)
                jmax = nv - 1
                h = [
                    hpsum.tile([P, 2, 512], F32, tag=f"hq{qc}", name=f"h{qc}_{s}")
                    for qc in range(2)
                ]
                sums = spsum.tile([P, 2], F32, tag="sums", name="sums")
                nc.tensor.matmul(
                    sums,
                    lhsT=zcol_sb,
                    rhs=zrow_sb[:, :2],
                    start=True,
                    stop=False,
                    skip_group_check=True,
                )
                # software pipeline: logits of j+1 are emitted before pv of j
                kt_t, v_t = load_visit(s, 0)
                lt_prev = logits(s, 0, kt_t)
                v_prev = v_t
                for j in range(1, nv):
                    kt_t, v_t = load_visit(s, j)
                    lt = logits(s, j, kt_t)
                    pv(s, j - 1, lt_prev, v_prev, h, sums, jmax)
                    lt_prev, v_prev = lt, v_t
                pv(s, jmax, lt_prev, v_prev, h, sums, jmax)

                # ---- epilogue ----------------------------------------
                g_bf = []
                for qc in range(2):
                    recip = epool.tile([P, 1], F32, tag="recip", name="recip")
                    nc.vector.reciprocal(out=recip, in_=sums[:, qc : qc + 1])
                    g32 = epool.tile([P, 2, 512], F32, tag="g32", name="g32")
                    nc.vector.tensor_scalar_mul(
                        out=g32, in0=h[qc], scalar1=recip
                    )
                    g = gpool.tile([P, 1024], BF16, tag=f"g{qc}", name=f"g{qc}")
                    nc.scalar.activation(
                        out=g, in_=g32.rearrange("p a b -> p (a b)"), func=AF.Silu
                    )
                    g_bf.append(g)
                # transpose G -> gt [d-part, m, 256]
                gt_sb = epool.tile([P, 8, 256], BF16, tag="gt", name="gt")
                for m in range(8):
                    for qc in range(2):
                        tp = tppsum.tile([P, 256], BF16, tag="tp", name="tp")
                        nc.tensor.transpose(
                            tp[:, :P],
                            g_bf[qc][:, m * P : (m + 1) * P],
                            ident_sb,
                        )
                        nc.vector.tensor_copy(
                            out=gt_sb[:, m, qc * P : (qc + 1) * P], in_=tp[:, :P]
                        )
                # output projection: O[q, d] via lhsT = gt chunks
                for qc in range(2):
                    op = hpsum.tile(
                        [P, 2, 512], F32, tag=f"hq{qc}", name=f"o{qc}_{s}"
                    )
                    for m in range(8):
                        for dh in range(2):
                            nc.tensor.matmul(
                                op[:, dh, :],
                                lhsT=gt_sb[:, m, qc * P : (qc + 1) * P],
                                rhs=wv2_bf[:, m, dh * 512 : (dh + 1) * 512],
                                start=(m == 0),
                                stop=(m == 7),
                            )
                    oo = epool.tile([P, 2, 512], F32, tag="oo", name="oo")
                    nc.vector.tensor_copy(out=oo, in_=op)
                    nc.sync.dma_start(
                        out=o_ext[s, qc], in_=oo.rearrange("p a b -> p (a b)")
                    )

        singles_ctx.__exit__(None, None, None)

    nc.finalize()
    return nc


_NC_CACHE = {}


def get_nc(b_dup=B_DUP):
    if b_dup not in _NC_CACHE:
        _NC_CACHE[b_dup] = build_kernel(b_dup)
    return _NC_CACHE[b_dup]


def build_masks():
    """Masks for the last 16 visits of each slot, selected per core by
    k = 2c + 16s - j: k>=1 all-visible, k==0 upper-left triangle, k==-1
    shifted triangle, k<=-2 fully masked (padded visit)."""
    p = np.arange(P)[:, None]
    u = np.arange(256)[None, :]
    m_ones = np.ones((P, 256), np.float32)
    m0 = (p <= u).astype(np.float32)
    m1 = (p <= u - P).astype(np.float32)
    m_zero = np.zeros((P, 256), np.float32)
    canon = np.stack([m_zero, m1, m0, m_ones]).astype(ml_dtypes.bfloat16)

    out = []
    for c in range(NCORES):
        sel = []
        for s in range(NSLOTS):
            for j in range(16 * s, 16 * (s + 1)):
                k = 2 * c + 16 * s - j
                sel.append(min(max(k, -2), 1) + 2)
        out.append(canon[np.array(sel, np.int64)])
    return out  # list of [64, 128, 256] bf16


def build_in_maps(x, wq, wk, wv1, wv2, b_dup=B_DUP):
    bf = ml_dtypes.bfloat16
    xT = np.ascontiguousarray(np.asarray(x, np.float32).T).astype(bf)
    masks = build_masks()
    xd = np.ascontiguousarray(xT[:, : b_dup * P])
    w = {
        "wq": np.asarray(wq, np.float32).astype(bf),
        "wk": np.asarray(wk, np.float32).astype(bf),
        "wv1": np.asarray(wv1, np.float32).astype(bf),
        "wv2": np.asarray(wv2, np.float32).astype(bf),
    }
    in_maps = []
    for c in range(NCORES):
        xq_c = np.concatenate(
            [xT[:, 256 * (c + 8 * s) : 256 * (c + 8 * s) + 256] for s in range(NSLOTS)],
            axis=1,
        )
        in_maps.append(
            {"xq": np.ascontiguousarray(xq_c), "xd": xd, "masks": masks[c], **w}
        )
    return in_maps


def assemble_out(results):
    out = np.empty((SEQ, D), np.float32)
    for c in range(NCORES):
        o = results[c]["o"]  # [4, 2, 128, 1024]
        for s in range(NSLOTS):
            r0 = 256 * (c + 8 * s)
            out[r0 : r0 + P, :] = o[s, 0]
            out[r0 + P : r0 + 256, :] = o[s, 1]
    return out


def kernel(x, wq, wk, wv1, wv2):
    in_maps = build_in_maps(x, wq, wk, wv1, wv2)
    nc = get_nc()
    res = run_bass_kernel_spmd(nc, in_maps, list(range(NCORES)))
    return assemble_out(res.results)



# revision 2
# speedup vs baseline: 1.3824x; 1.3824x over previous
"""Trainium2 Bass kernel for nn_MemoryAttention (causal single-head attention
with SiLU-gated output projection), sequence-parallel across 8 NeuronCores.

Strategy (per core c):
  - q rows owned: 4 slots of 256 rows: tile t = c + 8*s (strided assignment
    balances causal work; every core runs an identical instruction stream).
  - fp8e4 (DoubleRow, 2x PE rate) for the Q/K projections and the QK^T
    logits: softmax logits are tiny (~+-0.1 after 1/32 scaling), so ~5%
    quantization on q/k perturbs attention weights by well under 1%.
    wq/wk are pre-scaled by 64 host-side (w std 0.01 would land in the
    fp8 subnormal range); the exp() scale folds the 64*64 back out.
    V / PV / output projection stay bf16 (their quantization error would
    hit the output linearly).
  - Each core projects KT(fp8)/V(bf16) for its own tile of slot-level g,
    then immediately AllGathers that slot-level (4 small pipelined
    collectives instead of 2 big ones) so slot g's kv arrives while
    earlier slots compute. No duplicated local projection work.
  - Per (slot, kv-block) visit: LT[kv, q] via fp8 DoubleRow, PT =
    exp(LT*2^-17) (* mask for the last 16 visits of each slot), then PT
    q-chunks stationary for H[q, d] += P @ V and rowsums += P @ 1.
  - Slot epilogue: H / sums, SiLU, PE-transpose of G, output projection.
"""

import numpy as np
import ml_dtypes

import concourse.bass as bass
import concourse.tile as tile
from concourse import bacc, mybir
from concourse.bass_utils import run_bass_kernel_spmd
from concourse.masks import make_identity

P = 128
D = 1024
SEQ = 8192
NCORES = 8
NSLOTS = 4
WSCALE = 64.0
EXP_SCALE = 0.03125 / (WSCALE * WSCALE)

F32 = mybir.dt.float32
BF16 = mybir.dt.bfloat16
FP8 = mybir.dt.float8e4
AF = mybir.ActivationFunctionType
DR = mybir.MatmulPerfMode.DoubleRow


def build_kernel():
    nc = bacc.Bacc(None, target_bir_lowering=False, num_devices=NCORES)

    xb_ext = nc.declare_dram_parameter("xb", [D, D], BF16, isOutput=False)
    x8_ext = nc.declare_dram_parameter("x8", [D, D], FP8, isOutput=False)
    wq_ext = nc.declare_dram_parameter("wq", [D, D], FP8, isOutput=False)
    wk_ext = nc.declare_dram_parameter("wk", [D, D], FP8, isOutput=False)
    wv1_ext = nc.declare_dram_parameter("wv1", [D, D], BF16, isOutput=False)
    wv2_ext = nc.declare_dram_parameter("wv2", [D, D], BF16, isOutput=False)
    mask_ext = nc.declare_dram_parameter("masks", [64, P, 256], BF16, isOutput=False)
    o_ext = nc.declare_dram_parameter("o", [NSLOTS, 2, P, D], F32, isOutput=True)

    # per slot-level g: [0] = KT tile as fp8 bytes ([p, m(8), c(256)]),
    # [1],[2] = V blocks ([p, d]).  One AllGather per g, posted as soon as
    # this core's own tile (c + 8g) is projected.
    kv_local = nc.dram_tensor("kv_local", [NSLOTS, 3, P, D], BF16)
    kv_gath = nc.dram_tensor(
        "kv_gath", [NSLOTS, NCORES, 3, P, D], BF16, addr_space="Shared"
    )

    def wload(nc, pool, ext, tag, dt):
        t = pool.tile([P, 8, D], dt, tag=tag, name=tag)
        nc.sync.dma_start(out=t, in_=ext[:].rearrange("(sub p) s -> p sub s", p=P))
        return t

    with tile.TileContext(nc) as tc:
        singles_ctx = tc.tile_pool(name="singles", bufs=1)
        singles = singles_ctx.__enter__()

        ones_sb = singles.tile([P, 1], BF16)
        nc.vector.memset(ones_sb, 1.0)
        zcol_sb = singles.tile([1, P], BF16)
        nc.vector.memset(zcol_sb, 0.0)
        zrow_sb = singles.tile([1, 512], BF16)
        nc.vector.memset(zrow_sb, 0.0)
        ident_sb = singles.tile([P, P], BF16)
        make_identity(nc, ident_sb)
        qt_sb = singles.tile([P, 8, D], FP8)

        with (
            tc.tile_pool(name="projw", bufs=1) as projw,
            tc.tile_pool(name="projout", bufs=4) as projout,
            tc.tile_pool(name="ppsum", bufs=4, space="PSUM") as ppsum,
            tc.tile_pool(name="vpsum", bufs=4, space="PSUM") as vpsum,
        ):
            # chunked loads so the first projection matmuls start early
            x8_sb = projw.tile([P, 8, D], FP8, tag="x8", name="x8")
            x8_v = x8_ext[:].rearrange("(sub p) s -> p sub s", p=P)
            nc.sync.dma_start(out=x8_sb[:, :, :256], in_=x8_v[:, :, :256])
            wk8 = projw.tile([P, 8, D], FP8, tag="wk", name="wk")
            wk_v = wk_ext[:].rearrange("(sub p) s -> p sub s", p=P)
            for m in range(8):
                nc.sync.dma_start(
                    out=wk8[:, :, m * P : (m + 1) * P],
                    in_=wk_v[:, :, m * P : (m + 1) * P],
                )
            nc.sync.dma_start(out=x8_sb[:, :, 256:], in_=x8_v[:, :, 256:])
            xb_sb = wload(nc, projw, xb_ext, "xb", BF16)
            wv1_sb = wload(nc, projw, wv1_ext, "wv1", BF16)
            wq8 = wload(nc, projw, wq_ext, "wq", FP8)

            def kqt_proj(w8, cols, out_cb):
                # out[p(dout sub m), c] for c in cols; fp8 DoubleRow over d
                for m in range(8):
                    acc = ppsum.tile([P, 256], F32, tag="proj", name="kq")
                    for p4 in range(4):
                        nc.tensor.matmul(
                            acc,
                            lhsT=w8[:, 2 * p4 : 2 * p4 + 2, m * P : (m + 1) * P],
                            rhs=x8_sb[:, 2 * p4 : 2 * p4 + 2, cols],
                            start=(p4 == 0),
                            stop=(p4 == 3),
                            perf_mode=DR,
                        )
                    out_cb(m, acc)

            for g in range(NSLOTS):
                cols = slice(g * 256, (g + 1) * 256)
                kt_out = projout.tile([P, 8, 256], FP8, tag="kt_out", name="kto")
                kqt_proj(
                    wk8,
                    cols,
                    lambda m, acc: nc.vector.tensor_copy(out=kt_out[:, m, :], in_=acc),
                )
                nc.sync.dma_start(
                    out=kv_local[g, 0].bitcast(FP8),
                    in_=kt_out.rearrange("p m c -> p (m c)"),
                )
                for blk in range(2):
                    v_out = projout.tile([P, D], BF16, tag="v_out", name="vo")
                    accs = [
                        vpsum.tile([P, 512], F32, tag="vproj", name=f"vp{h2}")
                        for h2 in range(2)
                    ]
                    bc = slice(g * 256 + blk * P, g * 256 + (blk + 1) * P)
                    for sub in range(8):
                        for h2 in range(2):
                            nc.tensor.matmul(
                                accs[h2],
                                lhsT=xb_sb[:, sub, bc],
                                rhs=wv1_sb[:, sub, h2 * 512 : (h2 + 1) * 512],
                                start=(sub == 0),
                                stop=(sub == 7),
                            )
                    for h2 in range(2):
                        nc.vector.tensor_copy(
                            out=v_out[:, h2 * 512 : (h2 + 1) * 512], in_=accs[h2]
                        )
                    nc.sync.dma_start(out=kv_local[g, 1 + blk], in_=v_out)
                nc.gpsimd.collective_compute(
                    "AllGather",
                    mybir.AluOpType.bypass,
                    replica_groups=[list(range(NCORES))],
                    ins=[kv_local[g]],
                    outs=[kv_gath[g]],
                )

            for s in range(NSLOTS):
                cols = slice(s * 256, (s + 1) * 256)
                kqt_proj(
                    wq8,
                    cols,
                    lambda m, acc: nc.vector.tensor_copy(
                        out=qt_sb[:, m, s * 256 : (s + 1) * 256], in_=acc
                    ),
                )

        # ---- attention ----------------------------------------------------
        with (
            tc.tile_pool(name="asingles", bufs=1) as asingles,
            tc.tile_pool(name="vpool", bufs=5) as vpool,
            tc.tile_pool(name="ptpool", bufs=4) as ptpool,
            tc.tile_pool(name="mpool", bufs=3) as mpool,
            tc.tile_pool(name="epool", bufs=2) as epool,
            tc.tile_pool(name="gpool", bufs=2) as gpool,
            tc.tile_pool(name="ltpsum", bufs=2, space="PSUM") as ltpsum,
            tc.tile_pool(name="hpsum", bufs=1, space="PSUM") as hpsum,
            tc.tile_pool(name="spsum", bufs=1, space="PSUM") as spsum,
            tc.tile_pool(name="tppsum", bufs=1, space="PSUM") as tppsum,
        ):
            wv2_sb = wload(nc, asingles, wv2_ext, "wv2", BF16)

            def load_tile(t):
                kt_t = vpool.tile([P, 8, 256], FP8, tag="kt", name="kt_t")
                nc.sync.dma_start(
                    out=kt_t.rearrange("p m c -> p (m c)"),
                    in_=kv_gath[t // 8, t % 8, 0].bitcast(FP8),
                )
                v_t = vpool.tile([P, 2, D], BF16, tag="v", name="v_t")
                nc.sync.dma_start(
                    out=v_t,
                    in_=kv_gath[t // 8, t % 8, 1:3].rearrange("b p d -> p b d"),
                )
                return kt_t, v_t

            def logits(s, kt_t, b):
                lt = ltpsum.tile([P, 256], F32, tag="lt", name="lt")
                for p4 in range(4):
                    nc.tensor.matmul(
                        lt,
                        lhsT=kt_t[:, 2 * p4 : 2 * p4 + 2, b * P : (b + 1) * P],
                        rhs=qt_sb[:, 2 * p4 : 2 * p4 + 2, s * 256 : (s + 1) * 256],
                        start=(p4 == 0),
                        stop=(p4 == 3),
                        perf_mode=DR,
                    )
                return lt

            def pv(s, j, lt, v_t, b, h, sums, jmax):
                pt = ptpool.tile([P, 256], BF16, tag="pt", name="pt")
                nc.scalar.activation(out=pt, in_=lt, func=AF.Exp, scale=EXP_SCALE)
                if j >= 16 * s:
                    m_t = mpool.tile([P, 256], BF16, tag="m", name="m_t")
                    nc.sync.dma_start(out=m_t, in_=mask_ext[j])
                    nc.vector.tensor_mul(out=pt, in0=pt, in1=m_t)
                for qc in range(2):
                    lhsT = pt[:, qc * P : (qc + 1) * P]
                    for dh in range(2):
                        nc.tensor.matmul(
                            h[qc][:, dh, :],
                            lhsT=lhsT,
                            rhs=v_t[:, b, dh * 512 : (dh + 1) * 512],
                            start=(j == 0),
                            stop=(j == jmax),
                        )
                    nc.tensor.matmul(
                        sums[:, qc : qc + 1],
                        lhsT=lhsT,
                        rhs=ones_sb,
                        start=False,
                        stop=(j == jmax),
                        skip_group_check=True,
                    )

            for s in range(NSLOTS):
                nv = 16 * (s + 1)
                jmax = nv - 1
                h = [
                    hpsum.tile([P, 2, 512], F32, tag=f"hq{qc}", name=f"h{qc}_{s}")
                    for qc in range(2)
                ]
                sums = spsum.tile([P, 2], F32, tag="sums", name="sums")
                nc.tensor.matmul(
                    sums,
                    lhsT=zcol_sb,
                    rhs=zrow_sb[:, :2],
                    start=True,
                    stop=False,
                    skip_group_check=True,
                )
                # software pipeline: logits of j+1 are emitted before pv of j
                pend = None
                for t in range(8 * (s + 1)):
                    kt_t, v_t = load_tile(t)
                    for b in range(2):
                        j = 2 * t + b
                        lt = logits(s, kt_t, b)
                        if pend is not None:
                            pv(s, *pend, h, sums, jmax)
                        pend = (j, lt, v_t, b)
                pv(s, *pend, h, sums, jmax)

                # ---- epilogue ----------------------------------------
                g_bf = []
                for qc in range(2):
                    recip = epool.tile([P, 1], F32, tag="recip", name="recip")
                    nc.vector.reciprocal(out=recip, in_=sums[:, qc : qc + 1])
                    g32 = epool.tile([P, 2, 512], F32, tag="g32", name="g32")
                    nc.vector.tensor_scalar_mul(out=g32, in0=h[qc], scalar1=recip)
                    g = gpool.tile([P, D], BF16, tag=f"g{qc}", name=f"g{qc}")
                    nc.scalar.activation(
                        out=g, in_=g32.rearrange("p a b -> p (a b)"), func=AF.Silu
                    )
                    g_bf.append(g)
                # transpose G -> gt [d-part, m, 256]
                gt_sb = epool.tile([P, 8, 256], BF16, tag="gt", name="gt")
                for m in range(8):
                    for qc in range(2):
                        tp = tppsum.tile([P, 256], BF16, tag="tp", name="tp")
                        nc.tensor.transpose(
                            tp[:, :P],
                            g_bf[qc][:, m * P : (m + 1) * P],
                            ident_sb,
                        )
                        nc.vector.tensor_copy(
                            out=gt_sb[:, m, qc * P : (qc + 1) * P], in_=tp[:, :P]
                        )
                # output projection: O[q, d] via lhsT = gt chunks
                for qc in range(2):
                    op = hpsum.tile([P, 2, 512], F32, tag=f"hq{qc}", name=f"o{qc}_{s}")
                    for m in range(8):
                        for dh in range(2):
                            nc.tensor.matmul(
                                op[:, dh, :],
                                lhsT=gt_sb[:, m, qc * P : (qc + 1) * P],
                                rhs=wv2_sb[:, m, dh * 512 : (dh + 1) * 512],
                                start=(m == 0),
                                stop=(m == 7),
                            )
                    oo = epool.tile([P, 2, 512], F32, tag="oo", name="oo")
                    nc.vector.tensor_copy(out=oo, in_=op)
                    nc.sync.dma_start(
                        out=o_ext[s, qc], in_=oo.rearrange("p a b -> p (a b)")
                    )

        singles_ctx.__exit__(None, None, None)

    nc.finalize()
    return nc


_NC_CACHE = {}


def get_nc():
    if "nc" not in _NC_CACHE:
        _NC_CACHE["nc"] = build_kernel()
    return _NC_CACHE["nc"]


def build_masks():
    """Masks for the last 16 visits of each slot, selected per core by
    k = 2c + 16s - j: k>=1 all-visible, k==0 upper-left triangle, k==-1
    shifted triangle, k<=-2 fully masked (padded visit)."""
    p = np.arange(P)[:, None]
    u = np.arange(256)[None, :]
    m_ones = np.ones((P, 256), np.float32)
    m0 = (p <= u).astype(np.float32)
    m1 = (p <= u - P).astype(np.float32)
    m_zero = np.zeros((P, 256), np.float32)
    canon = np.stack([m_zero, m1, m0, m_ones]).astype(ml_dtypes.bfloat16)

    out = []
    for c in range(NCORES):
        sel = []
        for s in range(NSLOTS):
            for j in range(16 * s, 16 * (s + 1)):
                k = 2 * c + 16 * s - j
                sel.append(min(max(k, -2), 1) + 2)
        out.append(canon[np.array(sel, np.int64)])
    return out  # list of [64, 128, 256] bf16


def build_in_maps(x, wq, wk, wv1, wv2):
    bf = ml_dtypes.bfloat16
    f8 = ml_dtypes.float8_e4m3
    xT = np.ascontiguousarray(np.asarray(x, np.float32).T)
    masks = build_masks()
    w = {
        "wq": (np.asarray(wq, np.float32) * WSCALE).astype(f8),
        "wk": (np.asarray(wk, np.float32) * WSCALE).astype(f8),
        "wv1": np.asarray(wv1, np.float32).astype(bf),
        "wv2": np.asarray(wv2, np.float32).astype(bf),
    }
    in_maps = []
    for c in range(NCORES):
        xq_c = np.ascontiguousarray(
            np.concatenate(
                [
                    xT[:, 256 * (c + 8 * s) : 256 * (c + 8 * s) + 256]
                    for s in range(NSLOTS)
                ],
                axis=1,
            )
        )
        in_maps.append(
            {
                "xb": xq_c.astype(bf),
                "x8": xq_c.astype(f8),
                "masks": masks[c],
                **w,
            }
        )
    return in_maps


def assemble_out(results):
    out = np.empty((SEQ, D), np.float32)
    for c in range(NCORES):
        o = results[c]["o"]  # [4, 2, 128, 1024]
        for s in range(NSLOTS):
            r0 = 256 * (c + 8 * s)
            out[r0 : r0 + P, :] = o[s, 0]
            out[r0 + P : r0 + 256, :] = o[s, 1]
    return out


def kernel(x, wq, wk, wv1, wv2):
    in_maps = build_in_maps(x, wq, wk, wv1, wv2)
    nc = get_nc()
    res = run_bass_kernel_spmd(nc, in_maps, list(range(NCORES)))
    return assemble_out(res.results)


# revision 6
# speedup vs baseline: 1.4009x; 1.0134x over previous
"""Trainium2 Bass kernel for nn_MemoryAttention (causal single-head attention
with SiLU-gated output projection), sequence-parallel across 8 NeuronCores.

Strategy (per core c):
  - q rows owned: 4 slots of 256 rows: tile t = c + 8*s (strided assignment
    balances causal work; every core runs an identical instruction stream).
  - fp8e4 (DoubleRow, 2x PE rate) for the Q/K projections and the QK^T
    logits: softmax logits are tiny (~+-0.1 after 1/32 scaling), so ~5%
    quantization on q/k perturbs attention weights by well under 1%.
    wq/wk are pre-scaled by 64 host-side (w std 0.01 would land in the
    fp8 subnormal range); the exp() scale folds the 64*64 back out.
    V / PV / output projection stay bf16 (their quantization error would
    hit the output linearly).
  - Each core projects KT(fp8)/V(bf16) for its own tile of slot-level g,
    then immediately AllGathers that slot-level (4 small pipelined
    collectives instead of 2 big ones) so slot g's kv arrives while
    earlier slots compute. No duplicated local projection work.
  - Per (slot, kv-block) visit: LT[kv, q] via fp8 DoubleRow, PT =
    exp(LT*2^-17) (* mask for the last 16 visits of each slot), then PT
    q-chunks stationary for H[q, d] += P @ V and rowsums += P @ 1.
  - Slot epilogue: H / sums, SiLU, PE-transpose of G, output projection.
"""

import numpy as np
import ml_dtypes

import concourse.bass as bass
import concourse.tile as tile
from concourse import bacc, mybir
from concourse.bass_utils import run_bass_kernel_spmd
from concourse.masks import make_identity

P = 128
D = 1024
SEQ = 8192
NCORES = 8
NSLOTS = 4
WSCALE = 64.0
EXP_SCALE = 0.03125 / (WSCALE * WSCALE)

F32 = mybir.dt.float32
BF16 = mybir.dt.bfloat16
FP8 = mybir.dt.float8e4
AF = mybir.ActivationFunctionType
DR = mybir.MatmulPerfMode.DoubleRow


def build_kernel():
    nc = bacc.Bacc(None, target_bir_lowering=False, num_devices=NCORES)

    xb_ext = nc.declare_dram_parameter("xb", [D, D], BF16, isOutput=False)
    x8_ext = nc.declare_dram_parameter("x8", [D, D], FP8, isOutput=False)
    wq_ext = nc.declare_dram_parameter("wq", [D, D], FP8, isOutput=False)
    wk_ext = nc.declare_dram_parameter("wk", [D, D], FP8, isOutput=False)
    wv1_ext = nc.declare_dram_parameter("wv1", [D, D], BF16, isOutput=False)
    wv2_ext = nc.declare_dram_parameter("wv2", [D, D], BF16, isOutput=False)
    mask_ext = nc.declare_dram_parameter("masks", [64, P, 256], BF16, isOutput=False)
    o_ext = nc.declare_dram_parameter("o", [NSLOTS, 2, P, D], F32, isOutput=True)

    # per slot-level g: [0] = KT tile as fp8 bytes ([p, m(8), c(256)]),
    # [1],[2] = V blocks ([p, d]).  One AllGather per g, posted as soon as
    # this core's own tile (c + 8g) is projected.
    kv_local = nc.dram_tensor("kv_local", [NSLOTS, 3, P, D], BF16)
    kv_gath = nc.dram_tensor(
        "kv_gath", [NSLOTS, NCORES, 3, P, D], BF16, addr_space="Shared"
    )

    def wload(nc, pool, ext, tag, dt):
        t = pool.tile([P, 8, D], dt, tag=tag, name=tag)
        nc.sync.dma_start(out=t, in_=ext[:].rearrange("(sub p) s -> p sub s", p=P))
        return t

    with tile.TileContext(nc) as tc:
        singles_ctx = tc.tile_pool(name="singles", bufs=1)
        singles = singles_ctx.__enter__()

        ones_sb = singles.tile([P, 1], BF16)
        nc.vector.memset(ones_sb, 1.0)
        zcol_sb = singles.tile([1, P], BF16)
        nc.vector.memset(zcol_sb, 0.0)
        zrow_sb = singles.tile([1, 512], BF16)
        nc.vector.memset(zrow_sb, 0.0)
        ident_sb = singles.tile([P, P], BF16)
        make_identity(nc, ident_sb)
        qt_sb = singles.tile([P, 8, D], FP8)

        with (
            tc.tile_pool(name="projw", bufs=1) as projw,
            tc.tile_pool(name="projout", bufs=4) as projout,
            tc.tile_pool(name="ppsum", bufs=1, space="PSUM") as ppsum,
            tc.tile_pool(name="vpsum", bufs=4, space="PSUM") as vpsum,
        ):
            # sub-pair-chunked loads so the first DoubleRow matmuls (which
            # consume one sub-pair at a time) start after two small DMAs
            def pairload(pool, ext, tag, dt):
                t = pool.tile([P, 8, D], dt, tag=tag, name=tag)
                v = ext[:].rearrange("(sub p) s -> p sub s", p=P)
                for p4 in range(4):
                    nc.sync.dma_start(
                        out=t[:, 2 * p4 : 2 * p4 + 2, :], in_=v[:, 2 * p4 : 2 * p4 + 2, :]
                    )
                return t

            x8_v = x8_ext[:].rearrange("(sub p) s -> p sub s", p=P)
            wk_v = wk_ext[:].rearrange("(sub p) s -> p sub s", p=P)
            x8_sb = projw.tile([P, 8, D], FP8, tag="x8", name="x8")
            wk8 = projw.tile([P, 8, D], FP8, tag="wk", name="wk")
            for p4 in range(4):
                pr = slice(2 * p4, 2 * p4 + 2)
                nc.sync.dma_start(out=wk8[:, pr, :], in_=wk_v[:, pr, :])
                nc.sync.dma_start(out=x8_sb[:, pr, :], in_=x8_v[:, pr, :])
            xb_sb = pairload(projw, xb_ext, "xb", BF16)
            wv1_sb = pairload(projw, wv1_ext, "wv1", BF16)
            wq8 = pairload(projw, wq_ext, "wq", FP8)

            def kqt_proj(w8, cols, out_cb):
                # out[p(dout sub m), c] for c in cols; fp8 DoubleRow over d.
                # p4 outer so matmuls start as soon as sub-pair 0 arrives;
                # m in halves of 4 to stay within 4 PSUM banks.
                for half in range(2):
                    ms = range(4 * half, 4 * half + 4)
                    accs = {
                        m: ppsum.tile([P, 256], F32, tag=f"proj{m % 4}", name=f"kq{m}")
                        for m in ms
                    }
                    for p4 in range(4):
                        for m in ms:
                            nc.tensor.matmul(
                                accs[m],
                                lhsT=w8[:, 2 * p4 : 2 * p4 + 2, m * P : (m + 1) * P],
                                rhs=x8_sb[:, 2 * p4 : 2 * p4 + 2, cols],
                                start=(p4 == 0),
                                stop=(p4 == 3),
                                perf_mode=DR,
                            )
                    for m in ms:
                        out_cb(m, accs[m])

            for g in range(NSLOTS):
                cols = slice(g * 256, (g + 1) * 256)
                kt_out = projout.tile([P, 8, 256], FP8, tag="kt_out", name="kto")
                kqt_proj(
                    wk8,
                    cols,
                    lambda m, acc: nc.vector.tensor_copy(out=kt_out[:, m, :], in_=acc),
                )
                nc.sync.dma_start(
                    out=kv_local[g, 0].bitcast(FP8),
                    in_=kt_out.rearrange("p m c -> p (m c)"),
                )
                for blk in range(2):
                    v_out = projout.tile([P, D], BF16, tag="v_out", name="vo")
                    accs = [
                        vpsum.tile([P, 512], F32, tag="vproj", name=f"vp{h2}")
                        for h2 in range(2)
                    ]
                    bc = slice(g * 256 + blk * P, g * 256 + (blk + 1) * P)
                    for sub in range(8):
                        for h2 in range(2):
                            nc.tensor.matmul(
                                accs[h2],
                                lhsT=xb_sb[:, sub, bc],
                                rhs=wv1_sb[:, sub, h2 * 512 : (h2 + 1) * 512],
                                start=(sub == 0),
                                stop=(sub == 7),
                            )
                    for h2 in range(2):
                        nc.vector.tensor_copy(
                            out=v_out[:, h2 * 512 : (h2 + 1) * 512], in_=accs[h2]
                        )
                    nc.sync.dma_start(out=kv_local[g, 1 + blk], in_=v_out)
                nc.gpsimd.collective_compute(
                    "AllGather",
                    mybir.AluOpType.bypass,
                    replica_groups=[list(range(NCORES))],
                    ins=[kv_local[g]],
                    outs=[kv_gath[g]],
                )

            for s in range(NSLOTS):
                cols = slice(s * 256, (s + 1) * 256)
                kqt_proj(
                    wq8,
                    cols,
                    lambda m, acc: nc.vector.tensor_copy(
                        out=qt_sb[:, m, s * 256 : (s + 1) * 256], in_=acc
                    ),
                )

        # ---- attention ----------------------------------------------------
        with (
            tc.tile_pool(name="asingles", bufs=1) as asingles,
            tc.tile_pool(name="vpool", bufs=5) as vpool,
            tc.tile_pool(name="ptpool", bufs=4) as ptpool,
            tc.tile_pool(name="epool", bufs=2) as epool,
            tc.tile_pool(name="gpool", bufs=2) as gpool,
            tc.tile_pool(name="ltpsum", bufs=2, space="PSUM") as ltpsum,
            tc.tile_pool(name="hpsum", bufs=1, space="PSUM") as hpsum,
            tc.tile_pool(name="spsum", bufs=1, space="PSUM") as spsum,
            tc.tile_pool(name="tppsum", bufs=1, space="PSUM") as tppsum,
        ):
            wv2_sb = wload(nc, asingles, wv2_ext, "wv2", BF16)
            masks_sb = asingles.tile([P, 64, 256], BF16)
            nc.sync.dma_start(
                out=masks_sb, in_=mask_ext[:].rearrange("j p c -> p j c")
            )
            # slot-level-0 kv (tiles 0-7) stays SBUF-resident: it serves 64
            # of the 160 visits (first 16 of every slot) and decouples slots
            # 1-3's start from streaming DMA.
            kt0_sb = asingles.tile([P, 8, 8, 256], FP8)
            v0_sb = asingles.tile([P, 8, 2, D], BF16)
            for t in range(8):
                nc.sync.dma_start(
                    out=kt0_sb[:, t].rearrange("p m c -> p (m c)"),
                    in_=kv_gath[0, t, 0].bitcast(FP8),
                )
                nc.sync.dma_start(
                    out=v0_sb[:, t],
                    in_=kv_gath[0, t, 1:3].rearrange("b p d -> p b d"),
                )

            def load_tile(t):
                if t < 8:
                    return kt0_sb[:, t], v0_sb[:, t]
                kt_t = vpool.tile([P, 8, 256], FP8, tag="kt", name="kt_t")
                nc.sync.dma_start(
                    out=kt_t.rearrange("p m c -> p (m c)"),
                    in_=kv_gath[t // 8, t % 8, 0].bitcast(FP8),
                )
                v_t = vpool.tile([P, 2, D], BF16, tag="v", name="v_t")
                nc.sync.dma_start(
                    out=v_t,
                    in_=kv_gath[t // 8, t % 8, 1:3].rearrange("b p d -> p b d"),
                )
                return kt_t, v_t

            def logits(s, kt_t, b):
                lt = ltpsum.tile([P, 256], F32, tag="lt", name="lt")
                for p4 in range(4):
                    nc.tensor.matmul(
                        lt,
                        lhsT=kt_t[:, 2 * p4 : 2 * p4 + 2, b * P : (b + 1) * P],
                        rhs=qt_sb[:, 2 * p4 : 2 * p4 + 2, s * 256 : (s + 1) * 256],
                        start=(p4 == 0),
                        stop=(p4 == 3),
                        perf_mode=DR,
                    )
                return lt

            def pv(s, j, lt, v_t, b, h, sums, jmax):
                pt = ptpool.tile([P, 256], BF16, tag="pt", name="pt")
                nc.scalar.activation(out=pt, in_=lt, func=AF.Exp, scale=EXP_SCALE)
                if j >= 16 * s:
                    nc.vector.tensor_mul(out=pt, in0=pt, in1=masks_sb[:, j])
                for qc in range(2):
                    lhsT = pt[:, qc * P : (qc + 1) * P]
                    for dh in range(2):
                        nc.tensor.matmul(
                            h[qc][:, dh, :],
                            lhsT=lhsT,
                            rhs=v_t[:, b, dh * 512 : (dh + 1) * 512],
                            start=(j == 0),
                            stop=(j == jmax),
                        )
                    nc.tensor.matmul(
                        sums[:, qc : qc + 1],
                        lhsT=lhsT,
                        rhs=ones_sb,
                        start=False,
                        stop=(j == jmax),
                        skip_group_check=True,
                    )

            for s in range(NSLOTS):
                nv = 16 * (s + 1)
                jmax = nv - 1
                h = [
                    hpsum.tile([P, 2, 512], F32, tag=f"hq{qc}", name=f"h{qc}_{s}")
                    for qc in range(2)
                ]
                sums = spsum.tile([P, 2], F32, tag="sums", name="sums")
                nc.tensor.matmul(
                    sums,
                    lhsT=zcol_sb,
                    rhs=zrow_sb[:, :2],
                    start=True,
                    stop=False,
                    skip_group_check=True,
                )
                # software pipeline: logits of j+1 are emitted before pv of j
                pend = None
                for t in range(8 * (s + 1)):
                    kt_t, v_t = load_tile(t)
                    for b in range(2):
                        j = 2 * t + b
                        lt = logits(s, kt_t, b)
                        if pend is not None:
                            pv(s, *pend, h, sums, jmax)
                        pend = (j, lt, v_t, b)
                pv(s, *pend, h, sums, jmax)

                # ---- epilogue ----------------------------------------
                g_bf = []
                for qc in range(2):
                    recip = epool.tile([P, 1], F32, tag="recip", name="recip")
                    nc.vector.reciprocal(out=recip, in_=sums[:, qc : qc + 1])
                    g32 = epool.tile([P, 2, 512], F32, tag="g32", name="g32")
                    nc.vector.tensor_scalar_mul(out=g32, in0=h[qc], scalar1=recip)
                    g = gpool.tile([P, D], BF16, tag=f"g{qc}", name=f"g{qc}")
                    nc.scalar.activation(
                        out=g, in_=g32.rearrange("p a b -> p (a b)"), func=AF.Silu
                    )
                    g_bf.append(g)
                # transpose G -> gt [d-part, m, 256]
                gt_sb = epool.tile([P, 8, 256], BF16, tag="gt", name="gt")
                for m in range(8):
                    for qc in range(2):
                        tp = tppsum.tile([P, 256], BF16, tag="tp", name="tp")
                        nc.tensor.transpose(
                            tp[:, :P],
                            g_bf[qc][:, m * P : (m + 1) * P],
                            ident_sb,
                        )
                        nc.vector.tensor_copy(
                            out=gt_sb[:, m, qc * P : (qc + 1) * P], in_=tp[:, :P]
                        )
                # output projection: O[q, d] via lhsT = gt chunks
                for qc in range(2):
                    op = hpsum.tile([P, 2, 512], F32, tag=f"hq{qc}", name=f"o{qc}_{s}")
                    for m in range(8):
                        for dh in range(2):
                            nc.tensor.matmul(
                                op[:, dh, :],
                                lhsT=gt_sb[:, m, qc * P : (qc + 1) * P],
                                rhs=wv2_sb[:, m, dh * 512 : (dh + 1) * 512],
                                start=(m == 0),
                                stop=(m == 7),
                            )
                    oo = epool.tile([P, 2, 512], F32, tag="oo", name="oo")
                    nc.vector.tensor_copy(out=oo, in_=op)
                    nc.sync.dma_start(
                        out=o_ext[s, qc], in_=oo.rearrange("p a b -> p (a b)")
                    )

        singles_ctx.__exit__(None, None, None)

    nc.finalize()
    return nc


_NC_CACHE = {}


def get_nc():
    if "nc" not in _NC_CACHE:
        _NC_CACHE["nc"] = build_kernel()
    return _NC_CACHE["nc"]


def build_masks():
    """Masks for the last 16 visits of each slot, selected per core by
    k = 2c + 16s - j: k>=1 all-visible, k==0 upper-left triangle, k==-1
    shifted triangle, k<=-2 fully masked (padded visit)."""
    p = np.arange(P)[:, None]
    u = np.arange(256)[None, :]
    m_ones = np.ones((P, 256), np.float32)
    m0 = (p <= u).astype(np.float32)
    m1 = (p <= u - P).astype(np.float32)
    m_zero = np.zeros((P, 256), np.float32)
    canon = np.stack([m_zero, m1, m0, m_ones]).astype(ml_dtypes.bfloat16)

    out = []
    for c in range(NCORES):
        sel = []
        for s in range(NSLOTS):
            for j in range(16 * s, 16 * (s + 1)):
                k = 2 * c + 16 * s - j
                sel.append(min(max(k, -2), 1) + 2)
        out.append(canon[np.array(sel, np.int64)])
    return out  # list of [64, 128, 256] bf16


def build_in_maps(x, wq, wk, wv1, wv2):
    bf = ml_dtypes.bfloat16
    f8 = ml_dtypes.float8_e4m3
    xT = np.ascontiguousarray(np.asarray(x, np.float32).T)
    masks = build_masks()
    w = {
        "wq": (np.asarray(wq, np.float32) * WSCALE).astype(f8),
        "wk": (np.asarray(wk, np.float32) * WSCALE).astype(f8),
        "wv1": np.asarray(wv1, np.float32).astype(bf),
        "wv2": np.asarray(wv2, np.float32).astype(bf),
    }
    in_maps = []
    for c in range(NCORES):
        xq_c = np.ascontiguousarray(
            np.concatenate(
                [
                    xT[:, 256 * (c + 8 * s) : 256 * (c + 8 * s) + 256]
                    for s in range(NSLOTS)
                ],
                axis=1,
            )
        )
        in_maps.append(
            {
                "xb": xq_c.astype(bf),
                "x8": xq_c.astype(f8),
                "masks": masks[c],
                **w,
            }
        )
    return in_maps


def assemble_out(results):
    out = np.empty((SEQ, D), np.float32)
    for c in range(NCORES):
        o = results[c]["o"]  # [4, 2, 128, 1024]
        for s in range(NSLOTS):
            r0 = 256 * (c + 8 * s)
            out[r0 : r0 + P, :] = o[s, 0]
            out[r0 + P : r0 + 256, :] = o[s, 1]
    return out


def kernel(x, wq, wk, wv1, wv2):
    in_maps = build_in_maps(x, wq, wk, wv1, wv2)
    nc = get_nc()
    res = run_bass_kernel_spmd(nc, in_maps, list(range(NCORES)))
    return assemble_out(res.results)
